# revision 14
# baseline (speedup 1.0000x reference)
"""LoRA-injected 3x3 conv (MoE-routed adapters), Trainium2 Bass kernel.

Winograd F(2x2, 3x3) formulation:
 - Host: merge each sample's LoRA adapter into the base conv weight
   (W_eff = conv_w + scale*active * up @ down -- exact low-rank merge),
   Winograd-transform weights to U = G W G^T (16 taps) and inputs to
   V = B^T d B (16 taps, 32x32=1024 tiles/sample), both bf16. The conv
   becomes 16 independent [ci -> co] matmuls per tile set (2.25x fewer
   PE rows than direct conv). Bias rides as an extra contraction row:
   V row = 1.0, U row = bias on tap (1,1), whose output-transform
   coefficient is 1 for all 4 positions of a tile.
 - Device: per (sample, 512-tile chunk, co chunk): 4 tap-columns x
   4 tap-rows x 3 ci-chunks of accumulating bf16 matmuls into 8 PSUM
   banks. Output transform stage 1 (A^T M) is split Act/Pool/DVE so
   each tensor_tensor reads at most one PSUM operand: Act stages
   M1, M2 to SBUF; Pool forms M1+M2; DVE forms M1-M2 and the two
   PSUM-reading combines. Stage 2 ((.)A) runs on Pool writing four
   contiguous bf16 quadrant planes; the 2x2 spatial interleave and
   fp32 upcast happen on host.
 - First tile-chunk runs its matmuls ci-chunk-outer in column pairs so
   the PE starts after ~3.4MB of DMA instead of the full working set.
 - Batch sharded 2 samples/core across 8 cores.
"""

import sys

for _p in ("/opt/trn_rl_repo",):
    if _p not in sys.path:
        sys.path.insert(0, _p)

import numpy as np
import ml_dtypes

BF16 = ml_dtypes.bfloat16

B, CIN, COUT, H, W = 16, 320, 320, 64, 64
R, NUM_LORAS, LORA_STRIDE, SCALE = 4, 50, 4, 1.0
NCORES = 8
BLOC = B // NCORES            # samples per core
NT = 32 * 32                  # Winograd 2x2 tiles per sample
NTC = 512                     # tiles per device chunk
HWFLAT = H * W
CI_CHUNKS = [(0, 128), (128, 128), (256, 64)]   # ci contraction chunks
CO_CHUNKS = [(0, 128), (128, 128), (256, 64)]
BIAS_TAP = 5                  # tap (1,1): A-coeff 1 for all 4 positions

_NC_CACHE = {}


def _build_nc():
    import concourse.bacc as bacc
    import concourse.bass as bass
    import concourse.mybir as mybir
    from concourse import tile

    f32 = mybir.dt.float32
    bf16 = mybir.dt.bfloat16
    ADD = mybir.AluOpType.add
    SUB = mybir.AluOpType.subtract

    nc = bacc.Bacc(None, target_bir_lowering=False)

    # V: [sample, ci-chunk, tile-chunk, 128 rows, 16 taps * 512 tiles];
    # ci-chunk 2 row 64 = 1.0 on the bias tap (rows 65.. never read).
    v_d = nc.dram_tensor("v", [BLOC, 3, 2, 128, 16 * NTC], bf16, kind="ExternalInput")
    # U: [sample, ci-chunk, 128 rows, 16 taps * 320 co]
    u_d = nc.dram_tensor("u", [BLOC, 3, 128, 16 * COUT], bf16, kind="ExternalInput")
    # y: four 2x2-quadrant planes [pr*2+pc, co, tile], interleaved on host
    y_d = nc.dram_tensor("y", [BLOC, COUT, 4, NT], bf16, kind="ExternalOutput")

    with tile.TileContext(nc) as tc:
        with (
            tc.tile_pool(name="vio", bufs=2) as vpool,
            tc.tile_pool(name="uio", bufs=2) as upool,
            tc.tile_pool(name="t1", bufs=1) as tpool,
            tc.tile_pool(name="scr", bufs=1) as spool,
            tc.tile_pool(name="yst", bufs=2) as ypool,
            tc.tile_pool(name="acc", bufs=1, space=bass.MemorySpace.PSUM) as pspool,
        ):
            for b in range(BLOC):
                uts = []
                for ck in range(3):
                    ut = upool.tile([128, 16, COUT], bf16, tag=f"u{ck}")
                    kk = 65 if ck == 2 else 128
                    nc.gpsimd.dma_start(out=ut[:kk], in_=u_d[b, ck, :kk].rearrange(
                        "p (t c) -> p t c", c=COUT))
                    uts.append(ut)

                for tcix in range(2):
                    vts = []
                    for ck in range(3):
                        vt = vpool.tile([128, 16, NTC], bf16, tag=f"v{ck}")
                        kk = 65 if ck == 2 else 128
                        eng = [nc.sync, nc.sync, nc.gpsimd][ck]
                        eng.dma_start(out=vt[:kk], in_=v_d[b, ck, tcix, :kk].rearrange(
                            "p (t n) -> p t n", n=NTC))
                        vts.append(vt)

                    for cc, (o0, osz) in enumerate(CO_CHUNKS):
                        t1 = tpool.tile([128, 8, NTC], f32, tag="t1")
                        yt = ypool.tile([128, 4, NTC], bf16, tag="y")

                        def mm(c, tr, ck):
                            tap = 4 * tr + c
                            ps = pspool.tile([128, NTC], f32, tag=f"ps{c % 2}_{tr}")
                            kk = CI_CHUNKS[ck][1]
                            if ck == 2 and tap == BIAS_TAP:
                                kk = 65  # extra const-1 row carries bias
                            nc.tensor.matmul(
                                ps[:osz],
                                uts[ck][:kk, tap, o0 : o0 + osz],
                                vts[ck][:kk, tap, :],
                                start=(ck == 0),
                                stop=(ck == 2),
                            )
                            return ps

                        def stage1(c, pss):
                            # T1[0,c] = M0+M1+M2, T1[1,c] = M1-M2-M3;
                            # one PSUM operand per tensor_tensor.
                            c1 = spool.tile([128, NTC], bf16, tag=f"c1{c % 2}")
                            c2 = spool.tile([128, NTC], bf16, tag=f"c2{c % 2}")
                            s = spool.tile([128, NTC], bf16, tag=f"s{c % 2}")
                            dd = spool.tile([128, NTC], bf16, tag=f"d{c % 2}")
                            nc.scalar.copy(c1[:osz], pss[1][:osz])
                            nc.scalar.copy(c2[:osz], pss[2][:osz])
                            nc.gpsimd.tensor_tensor(
                                s[:osz], c1[:osz], c2[:osz], op=ADD)
                            nc.vector.tensor_tensor(
                                dd[:osz], c1[:osz], c2[:osz], op=SUB)
                            nc.vector.tensor_tensor(
                                t1[:osz, 0 + c], pss[0][:osz], s[:osz], op=ADD)
                            nc.vector.tensor_tensor(
                                t1[:osz, 4 + c], dd[:osz], pss[3][:osz], op=SUB)

                        first = b == 0 and tcix == 0 and cc == 0
                        if first:
                            # ci-chunk-outer in column pairs: first matmuls
                            # gate on (u0, v0) only, not the whole working set
                            for chalf in range(2):
                                cols = (2 * chalf, 2 * chalf + 1)
                                banks = {}
                                for ck in range(3):
                                    for c in cols:
                                        for tr in range(4):
                                            banks[(c, tr)] = mm(c, tr, ck)
                                for c in cols:
                                    stage1(c, [banks[(c, tr)] for tr in range(4)])
                        else:
                            for c in range(4):
                                pss = []
                                for tr in range(4):
                                    ps = None
                                    for ck in range(3):
                                        ps = mm(c, tr, ck)
                                    pss.append(ps)
                                stage1(c, pss)

                        # stage 2 on Pool: contiguous quadrant planes
                        for pr in range(2):
                            tmp = spool.tile([128, NTC], f32, tag=f"tmp{pr}")
                            nc.gpsimd.tensor_tensor(
                                tmp[:osz], t1[:osz, 4 * pr + 0], t1[:osz, 4 * pr + 1],
                                op=ADD)
                            nc.gpsimd.tensor_tensor(
                                yt[:osz, 2 * pr + 0], tmp[:osz], t1[:osz, 4 * pr + 2],
                                op=ADD)
                            nc.gpsimd.tensor_tensor(
                                tmp[:osz], t1[:osz, 4 * pr + 1], t1[:osz, 4 * pr + 2],
                                op=SUB)
                            nc.gpsimd.tensor_tensor(
                                yt[:osz, 2 * pr + 1], tmp[:osz], t1[:osz, 4 * pr + 3],
                                op=SUB)
                        nc.sync.dma_start(
                            out=y_d[b, o0 : o0 + osz, :, tcix * NTC : (tcix + 1) * NTC],
                            in_=yt[:osz],
                        )

    nc.compile()
    return nc


def _get_nc():
    if "nc" not in _NC_CACHE:
        _NC_CACHE["nc"] = _build_nc()
    return _NC_CACHE["nc"]


_G = np.array([[1, 0, 0], [0.5, 0.5, 0.5], [0.5, -0.5, 0.5], [0, 0, 1]], np.float32)


def _prep_inputs(x, conv_w, conv_b, down_w, up_w, lora_id):
    x = np.asarray(x, dtype=np.float32)
    conv_w = np.asarray(conv_w, dtype=np.float32)
    conv_b = np.asarray(conv_b, dtype=np.float32)
    down_w = np.asarray(down_w, dtype=np.float32)
    up_w = np.asarray(up_w, dtype=np.float32)
    idx = np.asarray(lora_id).astype(np.int64) // LORA_STRIDE
    active = (idx >= 0).astype(np.float32)
    safe = np.clip(idx, 0, NUM_LORAS - 1)

    # Exact LoRA merge: W_eff[b] = conv_w + scale*active_b * (up_b @ down_b)
    lora = np.matmul(up_w[safe], down_w[safe].reshape(B, R, -1))
    lora = lora.reshape(B, COUT, CIN, 3, 3)
    weff = conv_w[None] + (SCALE * active)[:, None, None, None, None] * lora

    # U[b, a, c, ci, co] = sum_{kh,kw} G[a,kh] G[c,kw] weff[b, co, ci, kh, kw]
    U = np.einsum("ab,cd,xoibd->xacio", _G, _G, weff, optimize=True)
    U = U.reshape(B, 16, CIN, COUT)

    # V via butterflies on xpad [B, ci, 66, 66]
    r = np.pad(x, ((0, 0), (0, 0), (1, 1), (1, 1)))
    t = np.stack([
        r[:, :, 0:64:2] - r[:, :, 2:66:2],
        r[:, :, 1:65:2] + r[:, :, 2:66:2],
        r[:, :, 2:66:2] - r[:, :, 1:65:2],
        r[:, :, 1:65:2] - r[:, :, 3:67:2],
    ], axis=1)  # [B, 4q, ci, 32, 66]
    V = np.stack([
        t[..., 0:64:2] - t[..., 2:66:2],
        t[..., 1:65:2] + t[..., 2:66:2],
        t[..., 2:66:2] - t[..., 1:65:2],
        t[..., 1:65:2] - t[..., 3:67:2],
    ], axis=2)  # [B, 4q, 4p, ci, 32, 32]
    V = V.reshape(B, 16, CIN, 2, NTC)   # tile-chunk split (i<16 | i>=16)

    v_all = np.zeros((B, 3, 2, 128, 16, NTC), dtype=BF16)
    u_all = np.zeros((B, 3, 128, 16, COUT), dtype=BF16)
    for ck, (k0, kk) in enumerate(CI_CHUNKS):
        v_all[:, ck, :, :kk] = V[:, :, k0 : k0 + kk].transpose(0, 3, 2, 1, 4).astype(BF16)
        u_all[:, ck, :kk] = U[:, :, k0 : k0 + kk].transpose(0, 2, 1, 3).astype(BF16)
    v_all[:, 2, :, 64, BIAS_TAP, :] = np.float32(1.0).astype(BF16)
    u_all[:, 2, 64, BIAS_TAP, :] = conv_b.astype(BF16)[None, :]

    v_all = v_all.reshape(B, 3, 2, 128, 16 * NTC)
    u_all = u_all.reshape(B, 3, 128, 16 * COUT)

    in_maps = [
        {
            "v": np.ascontiguousarray(v_all[c * BLOC : (c + 1) * BLOC]),
            "u": np.ascontiguousarray(u_all[c * BLOC : (c + 1) * BLOC]),
        }
        for c in range(NCORES)
    ]
    return in_maps


def run_device(in_maps, trace=False, tmpdir=None):
    from concourse.bass_utils import run_bass_kernel_spmd

    nc = _get_nc()
    return run_bass_kernel_spmd(
        nc, in_maps, list(range(NCORES)), trace=trace, tmpdir=tmpdir
    )


def _unpack_y(out):
    # y_d [BLOC, co, 4 planes, 1024 tiles] bf16 -> [B, co, 64, 64] fp32
    y = np.concatenate([out.results[c]["y"] for c in range(NCORES)], axis=0)
    y = y.astype(np.float32).reshape(B, COUT, 2, 2, 32, 32)
    y = y.transpose(0, 1, 4, 2, 5, 3).reshape(B, COUT, H, W)  # [co, i, pr, j, pc]
    return np.ascontiguousarray(y)


def kernel(x, conv_w, conv_b, down_w, up_w, lora_id):
    in_maps = _prep_inputs(x, conv_w, conv_b, down_w, up_w, lora_id)
    out = run_device(in_maps)
    return _unpack_y(out)


# revision 15
# speedup vs baseline: 1.3072x; 1.3072x over previous
"""LoRA-injected 3x3 conv (MoE-routed adapters), Trainium2 Bass kernel.

Winograd F(2x2, 3x3) formulation:
 - Host: merge each sample's LoRA adapter into the base conv weight
   (W_eff = conv_w + scale*active * up @ down -- exact low-rank merge),
   Winograd-transform weights to U = G W G^T (16 taps) and inputs to
   V = B^T d B (16 taps, 32x32=1024 tiles/sample), both bf16. The conv
   becomes 16 independent [ci -> co] matmuls per tile set (2.25x fewer
   PE rows than direct conv). Bias rides as an extra contraction row:
   V row = 1.0, U row = bias on tap (1,1), whose output-transform
   coefficient is 1 for all 4 positions of a tile.
 - Device: per (sample, 512-tile chunk, co chunk): 4 tap-columns x
   4 tap-rows x 3 ci-chunks of accumulating bf16 matmuls into 8 PSUM
   banks. Output transform stage 1 (A^T M) is split Act/Pool/DVE so
   each tensor_tensor reads at most one PSUM operand: Act stages
   M1, M2 to SBUF; Pool forms M1+M2; DVE forms M1-M2 and the two
   PSUM-reading combines. Stage 2 ((.)A) runs on Pool writing four
   contiguous bf16 quadrant planes; the 2x2 spatial interleave and
   fp32 upcast happen on host.
 - First tile-chunk runs its matmuls ci-chunk-outer in column pairs so
   the PE starts after ~3.4MB of DMA instead of the full working set.
 - Batch sharded 2 samples/core across 8 cores.
"""

import sys

for _p in ("/opt/trn_rl_repo",):
    if _p not in sys.path:
        sys.path.insert(0, _p)

import numpy as np
import ml_dtypes

BF16 = ml_dtypes.bfloat16

B, CIN, COUT, H, W = 16, 320, 320, 64, 64
R, NUM_LORAS, LORA_STRIDE, SCALE = 4, 50, 4, 1.0
NCORES = 8
BLOC = B // NCORES            # samples per core
NT = 32 * 32                  # Winograd 2x2 tiles per sample
NTC = 512                     # tiles per device chunk
HWFLAT = H * W
CI_CHUNKS = [(0, 128), (128, 128), (256, 64)]   # ci contraction chunks
CO_CHUNKS = [(0, 128), (128, 128), (256, 64)]
BIAS_TAP = 5                  # tap (1,1): A-coeff 1 for all 4 positions

_NC_CACHE = {}


def _build_nc():
    import concourse.bacc as bacc
    import concourse.bass as bass
    import concourse.mybir as mybir
    from concourse import tile

    f32 = mybir.dt.float32
    bf16 = mybir.dt.bfloat16
    ADD = mybir.AluOpType.add
    SUB = mybir.AluOpType.subtract

    nc = bacc.Bacc(None, target_bir_lowering=False)

    # V: [sample, ci-chunk, tile-chunk, 128 rows, 16 taps * 512 tiles];
    # ci-chunk 2 row 64 = 1.0 on the bias tap (rows 65.. never read).
    v_d = nc.dram_tensor("v", [BLOC, 3, 2, 128, 16 * NTC], bf16, kind="ExternalInput")
    # U: [sample, ci-chunk, 128 rows, 16 taps * 320 co]
    u_d = nc.dram_tensor("u", [BLOC, 3, 128, 16 * COUT], bf16, kind="ExternalInput")
    # t1: eight A^T M rows [4*pr+c, co, tile]; stage 2 of the output
    # transform ((.)A) and the 2x2 interleave run on host
    y_d = nc.dram_tensor("y", [BLOC, COUT, 8, NT], bf16, kind="ExternalOutput")

    with tile.TileContext(nc) as tc:
        with (
            tc.tile_pool(name="vio", bufs=2) as vpool,
            tc.tile_pool(name="uio", bufs=2) as upool,
            tc.tile_pool(name="t1", bufs=2) as tpool,
            tc.tile_pool(name="scr", bufs=1) as spool,
            tc.tile_pool(name="acc", bufs=1, space=bass.MemorySpace.PSUM) as pspool,
        ):
            for b in range(BLOC):
                uts = []
                for ck in range(3):
                    ut = upool.tile([128, 16, COUT], bf16, tag=f"u{ck}")
                    kk = 65 if ck == 2 else 128
                    nc.gpsimd.dma_start(out=ut[:kk], in_=u_d[b, ck, :kk].rearrange(
                        "p (t c) -> p t c", c=COUT))
                    uts.append(ut)

                for tcix in range(2):
                    vts = []
                    for ck in range(3):
                        vt = vpool.tile([128, 16, NTC], bf16, tag=f"v{ck}")
                        kk = 65 if ck == 2 else 128
                        eng = [nc.sync, nc.sync, nc.gpsimd][ck]
                        eng.dma_start(out=vt[:kk], in_=v_d[b, ck, tcix, :kk].rearrange(
                            "p (t n) -> p t n", n=NTC))
                        vts.append(vt)

                    for cc, (o0, osz) in enumerate(CO_CHUNKS):
                        t1 = tpool.tile([128, 8, NTC], bf16, tag="t1")

                        def mm(c, tr, ck):
                            tap = 4 * tr + c
                            ps = pspool.tile([128, NTC], f32, tag=f"ps{c % 2}_{tr}")
                            kk = CI_CHUNKS[ck][1]
                            if ck == 2 and tap == BIAS_TAP:
                                kk = 65  # extra const-1 row carries bias
                            nc.tensor.matmul(
                                ps[:osz],
                                uts[ck][:kk, tap, o0 : o0 + osz],
                                vts[ck][:kk, tap, :],
                                start=(ck == 0),
                                stop=(ck == 2),
                            )
                            return ps

                        def stage1(c, pss):
                            # T1[0,c] = M0+M1+M2, T1[1,c] = M1-M2-M3;
                            # one PSUM operand per tensor_tensor. Measured
                            # rates: DVE psum-op ~890ns, Act copy ~685ns,
                            # Pool bf16 ~700ns -- keep stage 2 off-device.
                            c1 = spool.tile([128, NTC], bf16, tag=f"c1{c % 2}")
                            c2 = spool.tile([128, NTC], bf16, tag=f"c2{c % 2}")
                            s = spool.tile([128, NTC], bf16, tag=f"s{c % 2}")
                            dd = spool.tile([128, NTC], bf16, tag=f"d{c % 2}")
                            nc.scalar.copy(c1[:osz], pss[1][:osz])
                            nc.scalar.copy(c2[:osz], pss[2][:osz])
                            nc.gpsimd.tensor_tensor(
                                s[:osz], c1[:osz], c2[:osz], op=ADD)
                            nc.vector.tensor_tensor(
                                dd[:osz], c1[:osz], c2[:osz], op=SUB)
                            nc.vector.tensor_tensor(
                                t1[:osz, 0 + c], pss[0][:osz], s[:osz], op=ADD)
                            nc.vector.tensor_tensor(
                                t1[:osz, 4 + c], dd[:osz], pss[3][:osz], op=SUB)

                        first = b == 0 and tcix == 0 and cc == 0
                        if first:
                            # ci-chunk-outer in column pairs: first matmuls
                            # gate on (u0, v0) only, not the whole working set
                            for chalf in range(2):
                                cols = (2 * chalf, 2 * chalf + 1)
                                banks = {}
                                for ck in range(3):
                                    for c in cols:
                                        for tr in range(4):
                                            banks[(c, tr)] = mm(c, tr, ck)
                                for c in cols:
                                    stage1(c, [banks[(c, tr)] for tr in range(4)])
                        else:
                            for c in range(4):
                                pss = []
                                for tr in range(4):
                                    ps = None
                                    for ck in range(3):
                                        ps = mm(c, tr, ck)
                                    pss.append(ps)
                                stage1(c, pss)

                        nc.sync.dma_start(
                            out=y_d[b, o0 : o0 + osz, :, tcix * NTC : (tcix + 1) * NTC],
                            in_=t1[:osz],
                        )

    nc.compile()
    return nc


def _get_nc():
    if "nc" not in _NC_CACHE:
        _NC_CACHE["nc"] = _build_nc()
    return _NC_CACHE["nc"]


_G = np.array([[1, 0, 0], [0.5, 0.5, 0.5], [0.5, -0.5, 0.5], [0, 0, 1]], np.float32)


def _prep_inputs(x, conv_w, conv_b, down_w, up_w, lora_id):
    x = np.asarray(x, dtype=np.float32)
    conv_w = np.asarray(conv_w, dtype=np.float32)
    conv_b = np.asarray(conv_b, dtype=np.float32)
    down_w = np.asarray(down_w, dtype=np.float32)
    up_w = np.asarray(up_w, dtype=np.float32)
    idx = np.asarray(lora_id).astype(np.int64) // LORA_STRIDE
    active = (idx >= 0).astype(np.float32)
    safe = np.clip(idx, 0, NUM_LORAS - 1)

    # Exact LoRA merge: W_eff[b] = conv_w + scale*active_b * (up_b @ down_b)
    lora = np.matmul(up_w[safe], down_w[safe].reshape(B, R, -1))
    lora = lora.reshape(B, COUT, CIN, 3, 3)
    weff = conv_w[None] + (SCALE * active)[:, None, None, None, None] * lora

    # U[b, a, c, ci, co] = sum_{kh,kw} G[a,kh] G[c,kw] weff[b, co, ci, kh, kw]
    U = np.einsum("ab,cd,xoibd->xacio", _G, _G, weff, optimize=True)
    U = U.reshape(B, 16, CIN, COUT)

    # V via butterflies on xpad [B, ci, 66, 66]
    r = np.pad(x, ((0, 0), (0, 0), (1, 1), (1, 1)))
    t = np.stack([
        r[:, :, 0:64:2] - r[:, :, 2:66:2],
        r[:, :, 1:65:2] + r[:, :, 2:66:2],
        r[:, :, 2:66:2] - r[:, :, 1:65:2],
        r[:, :, 1:65:2] - r[:, :, 3:67:2],
    ], axis=1)  # [B, 4q, ci, 32, 66]
    V = np.stack([
        t[..., 0:64:2] - t[..., 2:66:2],
        t[..., 1:65:2] + t[..., 2:66:2],
        t[..., 2:66:2] - t[..., 1:65:2],
        t[..., 1:65:2] - t[..., 3:67:2],
    ], axis=2)  # [B, 4q, 4p, ci, 32, 32]
    V = V.reshape(B, 16, CIN, 2, NTC)   # tile-chunk split (i<16 | i>=16)

    v_all = np.zeros((B, 3, 2, 128, 16, NTC), dtype=BF16)
    u_all = np.zeros((B, 3, 128, 16, COUT), dtype=BF16)
    for ck, (k0, kk) in enumerate(CI_CHUNKS):
        v_all[:, ck, :, :kk] = V[:, :, k0 : k0 + kk].transpose(0, 3, 2, 1, 4).astype(BF16)
        u_all[:, ck, :kk] = U[:, :, k0 : k0 + kk].transpose(0, 2, 1, 3).astype(BF16)
    v_all[:, 2, :, 64, BIAS_TAP, :] = np.float32(1.0).astype(BF16)
    u_all[:, 2, 64, BIAS_TAP, :] = conv_b.astype(BF16)[None, :]

    v_all = v_all.reshape(B, 3, 2, 128, 16 * NTC)
    u_all = u_all.reshape(B, 3, 128, 16 * COUT)

    in_maps = [
        {
            "v": np.ascontiguousarray(v_all[c * BLOC : (c + 1) * BLOC]),
            "u": np.ascontiguousarray(u_all[c * BLOC : (c + 1) * BLOC]),
        }
        for c in range(NCORES)
    ]
    return in_maps


def run_device(in_maps, trace=False, tmpdir=None):
    from concourse.bass_utils import run_bass_kernel_spmd

    nc = _get_nc()
    return run_bass_kernel_spmd(
        nc, in_maps, list(range(NCORES)), trace=trace, tmpdir=tmpdir
    )


def _unpack_y(out):
    # y_d [BLOC, co, 8 = (pr, c), 1024 tiles] bf16 holds T1 = A^T M;
    # apply stage 2 (columns) in fp32 and interleave 2x2 quadrants.
    t = np.concatenate([out.results[c]["y"] for c in range(NCORES)], axis=0)
    t = t.astype(np.float32).reshape(B, COUT, 2, 4, 32, 32)  # [b,co,pr,c,i,j]
    y0 = t[:, :, :, 0] + t[:, :, :, 1] + t[:, :, :, 2]       # [b,co,pr,i,j]
    y1 = t[:, :, :, 1] - t[:, :, :, 2] - t[:, :, :, 3]
    y = np.empty((B, COUT, 32, 2, 32, 2), np.float32)        # [b,co,i,pr,j,pc]
    y[..., 0] = y0.transpose(0, 1, 3, 2, 4)
    y[..., 1] = y1.transpose(0, 1, 3, 2, 4)
    return np.ascontiguousarray(y.reshape(B, COUT, H, W))


def kernel(x, conv_w, conv_b, down_w, up_w, lora_id):
    in_maps = _prep_inputs(x, conv_w, conv_b, down_w, up_w, lora_id)
    out = run_device(in_maps)
    return _unpack_y(out)
